# revision 6
# baseline (speedup 1.0000x reference)
"""CenterLoss forward on 8 Trainium2 NeuronCores.

Full inputs in, full outputs out.  Strategy (expert-parallel style, per the
row-sharded centers table):
  - centers [100000, 512] is sharded row-wise: core c owns rows
    [c*12500, (c+1)*12500).
  - Host routes each sample i to the core owning y[i], dedups indices per
    core (summing batch rows for duplicate classes), so the device scatter
    is a plain unique-row write.
  - Each core: bulk-copies its shard to the output table (the memory-bound
    bulk of the op), gathers the unique rows, computes
        new_row = c + ALPHA * (bsum - cnt * c)
    and the loss partials
        sum_f (cnt*c - 2*bsum) * c        (per unique row)
        sum_f b*b                         (over a row-slice of batch)
    then scatters the updated rows into the output shard.
  - Host: loss = LAMBDA/B * (sum of all partials + sum_i |b_i|^2 term
    folded in on-device), concat shards.
"""

import sys

for _p in ("/opt/trn_rl_repo",):
    if _p not in sys.path:
        sys.path.insert(0, _p)

import numpy as np

from concourse import bacc, bass, mybir, tile
from concourse.bass import IndirectOffsetOnAxis
from concourse.bass_utils import run_bass_kernel_spmd

M = 8  # cores
NUM_CLASSES = 100000
E = 512
B = 4096
R = NUM_CLASSES // M  # 12500 rows per core
BS = B // M  # 512 batch rows per core for the |b|^2 term
ALPHA = 0.1
LAMBDA = 0.01
P = 128
F32 = mybir.dt.float32
I32 = mybir.dt.int32

_BUILD_CACHE: dict[int, "bass.Bass"] = {}


def _build(T: int) -> "bass.Bass":
    """Build the per-core kernel for T tiles (T*128 unique-row capacity)."""
    nc = bacc.Bacc(None, target_bir_lowering=False)
    centers_in = nc.dram_tensor("centers_in", [R, E], F32, kind="ExternalInput")
    idx_in = nc.dram_tensor("idx_in", [P, T], I32, kind="ExternalInput")
    cnt_in = nc.dram_tensor("cnt_in", [P, T], F32, kind="ExternalInput")
    bsum_in = nc.dram_tensor("bsum_in", [T * P, E], F32, kind="ExternalInput")
    bslice_in = nc.dram_tensor("bslice_in", [BS, E], F32, kind="ExternalInput")
    centers_out = nc.dram_tensor("centers_out", [R, E], F32, kind="ExternalOutput")
    loss_out = nc.dram_tensor("loss_out", [P, 1], F32, kind="ExternalOutput")

    n_bt = BS // P  # batch tiles
    add = mybir.AluOpType.add
    mult = mybir.AluOpType.mult
    subtract = mybir.AluOpType.subtract

    with tile.TileContext(nc) as tc:
        with (
            tc.tile_pool(name="sbuf", bufs=1) as pool,
            tc.tile_pool(name="accp", bufs=1) as accp,
        ):
            acc = accp.tile([P, 1], F32)

            # --- batch-slice |b|^2 partials (first one initializes acc) ---
            for t in range(n_bt):
                b = pool.tile([P, E], F32, tag=f"bt{t}")
                nc.sync.dma_start(out=b[:], in_=bslice_in[t * P : (t + 1) * P, :])
                prod = pool.tile([P, E], F32, tag=f"prod{t}")
                if t == 0:
                    nc.vector.scalar_tensor_tensor(
                        out=prod[:], in0=b[:], scalar=1.0, in1=b[:],
                        op0=mult, op1=mult, accum_out=acc[:],
                    )
                else:
                    part = pool.tile([P, 1], F32, tag=f"part{t}")
                    nc.vector.scalar_tensor_tensor(
                        out=prod[:], in0=b[:], scalar=1.0, in1=b[:],
                        op0=mult, op1=mult, accum_out=part[:],
                    )
                    nc.vector.tensor_tensor(out=acc[:], in0=acc[:], in1=part[:], op=add)

            # --- unique-row metadata loads (small, single DMAs) ---
            idx_sb = pool.tile([P, T], I32, tag="idx")
            nc.sync.dma_start(out=idx_sb[:], in_=idx_in[:])
            cnt_sb = pool.tile([P, T], F32, tag="cnt")
            nc.sync.dma_start(out=cnt_sb[:], in_=cnt_in[:])

            # --- per-tile gather / compute ---
            newcs = []
            for t in range(T):
                c = pool.tile([P, E], F32, tag=f"c{t}")
                nc.gpsimd.indirect_dma_start(
                    out=c[:],
                    out_offset=None,
                    in_=centers_in[:],
                    in_offset=IndirectOffsetOnAxis(ap=idx_sb[:, t : t + 1], axis=0),
                )
                s = pool.tile([P, E], F32, tag=f"s{t}")
                nc.sync.dma_start(out=s[:], in_=bsum_in[t * P : (t + 1) * P, :])

                q = pool.tile([P, E], F32, tag=f"q{t}")
                nc.vector.tensor_scalar_mul(out=q[:], in0=c[:], scalar1=cnt_sb[:, t : t + 1])
                # r = q - 2*s
                r = pool.tile([P, E], F32, tag=f"r{t}")
                nc.vector.scalar_tensor_tensor(
                    out=r[:], in0=s[:], scalar=-2.0, in1=q[:], op0=mult, op1=add,
                )
                # loss partial: sum_f r * c
                prod2 = pool.tile([P, E], F32, tag=f"prod2{t}")
                part2 = pool.tile([P, 1], F32, tag=f"part2{t}")
                nc.vector.scalar_tensor_tensor(
                    out=prod2[:], in0=r[:], scalar=1.0, in1=c[:],
                    op0=mult, op1=mult, accum_out=part2[:],
                )
                nc.vector.tensor_tensor(out=acc[:], in0=acc[:], in1=part2[:], op=add)
                # new_c = c + ALPHA * (s - q)
                d = pool.tile([P, E], F32, tag=f"d{t}")
                nc.vector.tensor_tensor(out=d[:], in0=s[:], in1=q[:], op=subtract)
                newc = pool.tile([P, E], F32, tag=f"newc{t}")
                nc.vector.scalar_tensor_tensor(
                    out=newc[:], in0=d[:], scalar=ALPHA, in1=c[:], op0=mult, op1=add,
                )
                newcs.append(newc)

            # --- bulk shard copy (the memory-bound part) on the other HWDGE ---
            CHUNK = 3125
            for i in range(0, R, CHUNK):
                j = min(i + CHUNK, R)
                nc.scalar.dma_start(out=centers_out[i:j, :], in_=centers_in[i:j, :])

            # --- scatter updated rows (Tile orders these after the copy) ---
            for t in range(T):
                nc.gpsimd.indirect_dma_start(
                    out=centers_out[:],
                    out_offset=IndirectOffsetOnAxis(ap=idx_sb[:, t : t + 1], axis=0),
                    in_=newcs[t][:],
                    in_offset=None,
                )

            nc.sync.dma_start(out=loss_out[:], in_=acc[:])
    nc.finalize()
    return nc


def _route(y: np.ndarray, batch: np.ndarray):
    """Per-core routing: unique local rows, counts, summed batch rows."""
    owner = y // R
    local = (y % R).astype(np.int64)
    per_core = []
    max_u = 1
    for c in range(M):
        m = owner == c
        loc = local[m]
        rows = batch[m]
        if loc.size:
            uniq, inv, cnts = np.unique(loc, return_inverse=True, return_counts=True)
            bsums = np.zeros((uniq.size, E), np.float32)
            np.add.at(bsums, inv, rows)
        else:
            uniq = np.zeros((0,), np.int64)
            cnts = np.zeros((0,), np.int64)
            bsums = np.zeros((0, E), np.float32)
        per_core.append((uniq, cnts, bsums))
        max_u = max(max_u, uniq.size)
    T = -(-max_u // P)  # tiles of 128
    CU = T * P
    in_maps = []
    for c in range(M):
        uniq, cnts, bsums = per_core[c]
        # pad with an unused row: scatter writes it back unchanged
        free = np.setdiff1d(np.arange(uniq.size + 1, dtype=np.int64), uniq)[0]
        idx = np.full((CU,), free, np.int32)
        idx[: uniq.size] = uniq
        cnt = np.zeros((CU,), np.float32)
        cnt[: uniq.size] = cnts
        bsum = np.zeros((CU, E), np.float32)
        bsum[: uniq.size] = bsums
        in_maps.append(
            {
                "idx_in": np.ascontiguousarray(idx.reshape(T, P).T),
                "cnt_in": np.ascontiguousarray(cnt.reshape(T, P).T),
                "bsum_in": bsum,
            }
        )
    return T, in_maps


def kernel(y, batch, centers):
    y = np.asarray(y)
    batch = np.ascontiguousarray(np.asarray(batch, dtype=np.float32))
    centers = np.ascontiguousarray(np.asarray(centers, dtype=np.float32))
    y64 = y.astype(np.int64)

    T, in_maps = _route(y64, batch)
    for c in range(M):
        in_maps[c]["centers_in"] = centers[c * R : (c + 1) * R]
        in_maps[c]["bslice_in"] = batch[c * BS : (c + 1) * BS]

    nc = _BUILD_CACHE.get(T)
    if nc is None:
        nc = _build(T)
        _BUILD_CACHE[T] = nc

    res = run_bass_kernel_spmd(nc, in_maps, list(range(M))).results

    new_centers = np.concatenate([res[c]["centers_out"] for c in range(M)], axis=0)
    total = np.float64(0.0)
    for c in range(M):
        total += np.asarray(res[c]["loss_out"], dtype=np.float64).sum()
    loss = np.asarray(LAMBDA * total / B, dtype=np.float32)
    return loss, new_centers


# revision 12
# speedup vs baseline: 1.0198x; 1.0198x over previous
"""CenterLoss forward on 8 Trainium2 NeuronCores.

Full inputs in, full outputs out.  Strategy (expert-parallel style, per the
row-sharded centers table):
  - centers [100000, 512] is sharded row-wise: core c owns rows
    [c*12500, (c+1)*12500).
  - Host routes each sample i to the core owning y[i], dedups indices per
    core (summing batch rows for duplicate classes), so the device scatter
    is a plain unique-row write.
  - Each core: bulk-copies its shard to the output table (the memory-bound
    bulk of the op), gathers the unique rows, computes
        new_row = c + ALPHA * (bsum - cnt * c)
    and the loss partials
        sum_f (cnt*c - 2*bsum) * c        (per unique row)
        sum_f b*b                         (over a row-slice of batch)
    then scatters the updated rows into the output shard.
  - The output shard is split into K segment tensors so the scatter for
    segment k only waits on segment k's copy (overlaps later copies,
    removing the serial scatter tail).  Copies alternate between the two
    HWDGE rings (sync + scalar) for bandwidth.
  - Host: loss = LAMBDA/B * (sum of partials), concat segments.
"""

import sys

for _p in ("/opt/trn_rl_repo",):
    if _p not in sys.path:
        sys.path.insert(0, _p)

import numpy as np

from concourse import bacc, bass, mybir, tile
from concourse.bass import IndirectOffsetOnAxis
from concourse.bass_utils import run_bass_kernel_spmd

M = 8  # cores
NUM_CLASSES = 100000
E = 512
B = 4096
R = NUM_CLASSES // M  # 12500 rows per core
K = 4  # output segments per core
RS = R // K  # 3125 rows per segment
BS = B // M  # 512 batch rows per core for the |b|^2 term
ALPHA = 0.1
LAMBDA = 0.01
P = 128
F32 = mybir.dt.float32
I32 = mybir.dt.int32

_BUILD_CACHE: dict[int, "bass.Bass"] = {}


def _build(T: int) -> "bass.Bass":
    """Per-core kernel; T gather tiles of 128 unique rows per segment."""
    nc = bacc.Bacc(None, target_bir_lowering=False)
    centers_in = nc.dram_tensor("centers_in", [R, E], F32, kind="ExternalInput")
    # gather indices are core-local [0,R); scatter indices segment-local [0,RS)
    idxg_in = nc.dram_tensor("idxg_in", [P, K * T], I32, kind="ExternalInput")
    idxs_in = nc.dram_tensor("idxs_in", [P, K * T], I32, kind="ExternalInput")
    cnt_in = nc.dram_tensor("cnt_in", [P, K * T], F32, kind="ExternalInput")
    # wrapped layout: bsum_in[p, (k*T+t)*E + e] = bsum row (k*T+t)*P + p
    bsum_in = nc.dram_tensor("bsum_in", [P, K * T * E], F32, kind="ExternalInput")
    bslice_in = nc.dram_tensor("bslice_in", [BS, E], F32, kind="ExternalInput")
    outs = [
        nc.dram_tensor(f"out{k}", [RS, E], F32, kind="ExternalOutput") for k in range(K)
    ]
    loss_out = nc.dram_tensor("loss_out", [P, 1], F32, kind="ExternalOutput")

    n_bt = BS // P  # batch tiles
    add = mybir.AluOpType.add
    mult = mybir.AluOpType.mult
    subtract = mybir.AluOpType.subtract

    with tile.TileContext(nc) as tc:
        with (
            tc.tile_pool(name="sbuf", bufs=1) as pool,
            tc.tile_pool(name="accp", bufs=1) as accp,
        ):
            acc = accp.tile([P, 1], F32)

            # --- small metadata loads first (head of the sync HWDGE ring) ---
            idxg_sb = pool.tile([P, K * T], I32, tag="idxg")
            nc.sync.dma_start(out=idxg_sb[:], in_=idxg_in[:])
            idxs_sb = pool.tile([P, K * T], I32, tag="idxs")
            nc.sync.dma_start(out=idxs_sb[:], in_=idxs_in[:])
            cnt_sb = pool.tile([P, K * T], F32, tag="cnt")
            nc.sync.dma_start(out=cnt_sb[:], in_=cnt_in[:])

            # --- batch-slice |b|^2 partials (first one initializes acc) ---
            for t in range(n_bt):
                b = pool.tile([P, E], F32, tag=f"bt{t}")
                nc.sync.dma_start(out=b[:], in_=bslice_in[t * P : (t + 1) * P, :])
                prod = pool.tile([P, E], F32, tag=f"prod{t}")
                if t == 0:
                    nc.vector.scalar_tensor_tensor(
                        out=prod[:], in0=b[:], scalar=1.0, in1=b[:],
                        op0=mult, op1=mult, accum_out=acc[:],
                    )
                else:
                    part = pool.tile([P, 1], F32, tag=f"part{t}")
                    nc.vector.scalar_tensor_tensor(
                        out=prod[:], in0=b[:], scalar=1.0, in1=b[:],
                        op0=mult, op1=mult, accum_out=part[:],
                    )
                    nc.vector.tensor_tensor(out=acc[:], in0=acc[:], in1=part[:], op=add)

            # --- per-segment gather (one batched indirect DMA) + compute ---
            newc_sbs = []
            for k in range(K):
                c_sb = pool.tile([P, T * E], F32, tag=f"c{k}")
                for t in range(T):
                    g = k * T + t
                    nc.gpsimd.indirect_dma_start(
                        out=c_sb[:, t * E : (t + 1) * E],
                        out_offset=None,
                        in_=centers_in[:],
                        in_offset=IndirectOffsetOnAxis(
                            ap=idxg_sb[:, g : g + 1], axis=0
                        ),
                    )
                bs_sb = pool.tile([P, T * E], F32, tag=f"s{k}")
                nc.sync.dma_start(
                    out=bs_sb[:], in_=bsum_in[:, k * T * E : (k + 1) * T * E]
                )
                newc_sb = pool.tile([P, T * E], F32, tag=f"newc{k}")
                newc_sbs.append(newc_sb)
                for t in range(T):
                    g = k * T + t
                    sl = slice(t * E, (t + 1) * E)
                    c = c_sb[:, sl]
                    s = bs_sb[:, sl]
                    q = pool.tile([P, E], F32, tag=f"q{g}")
                    nc.vector.tensor_scalar_mul(
                        out=q[:], in0=c, scalar1=cnt_sb[:, g : g + 1]
                    )
                    # r = q - 2*s
                    r = pool.tile([P, E], F32, tag=f"r{g}")
                    nc.vector.scalar_tensor_tensor(
                        out=r[:], in0=s, scalar=-2.0, in1=q[:], op0=mult, op1=add,
                    )
                    # loss partial: sum_f r * c
                    prod2 = pool.tile([P, E], F32, tag=f"prod2{g}")
                    part2 = pool.tile([P, 1], F32, tag=f"part2{g}")
                    nc.vector.scalar_tensor_tensor(
                        out=prod2[:], in0=r[:], scalar=1.0, in1=c,
                        op0=mult, op1=mult, accum_out=part2[:],
                    )
                    nc.vector.tensor_tensor(
                        out=acc[:], in0=acc[:], in1=part2[:], op=add
                    )
                    # new_c = c + ALPHA * (s - q)
                    d = pool.tile([P, E], F32, tag=f"d{g}")
                    nc.vector.tensor_tensor(out=d[:], in0=s, in1=q[:], op=subtract)
                    nc.vector.scalar_tensor_tensor(
                        out=newc_sb[:, sl], in0=d[:], scalar=ALPHA, in1=c,
                        op0=mult, op1=add,
                    )

            # --- bulk segment copies: halves on the two HWDGE rings ---
            H = RS // 2
            for k in range(K):
                nc.sync.dma_start(
                    out=outs[k][:H, :], in_=centers_in[k * RS : k * RS + H, :]
                )
                nc.scalar.dma_start(
                    out=outs[k][H:, :], in_=centers_in[k * RS + H : (k + 1) * RS, :]
                )

            # --- scatter updated rows per segment (waits only on its copy) ---
            for k in range(K):
                for t in range(T):
                    g = k * T + t
                    nc.gpsimd.indirect_dma_start(
                        out=outs[k][:],
                        out_offset=IndirectOffsetOnAxis(
                            ap=idxs_sb[:, g : g + 1], axis=0
                        ),
                        in_=newc_sbs[k][:, t * E : (t + 1) * E],
                        in_offset=None,
                    )

            nc.sync.dma_start(out=loss_out[:], in_=acc[:])
    nc.finalize()
    return nc


def _route(y: np.ndarray, batch: np.ndarray):
    """Route samples to (core, segment); dedup; build padded device arrays."""
    owner = y // R
    local = (y % R).astype(np.int64)
    seg = local // RS
    per_bin: list[list] = []
    max_u = 1
    for c in range(M):
        for k in range(K):
            m = (owner == c) & (seg == k)
            loc = local[m] - k * RS  # segment-local
            rows = batch[m]
            if loc.size:
                uniq, inv, cnts = np.unique(
                    loc, return_inverse=True, return_counts=True
                )
                bsums = np.zeros((uniq.size, E), np.float32)
                np.add.at(bsums, inv, rows)
            else:
                uniq = np.zeros((0,), np.int64)
                cnts = np.zeros((0,), np.int64)
                bsums = np.zeros((0, E), np.float32)
            per_bin.append((uniq, cnts, bsums))
            max_u = max(max_u, uniq.size)
    T = -(-max_u // P)  # tiles of 128 per segment
    CU = T * P
    in_maps = []
    for c in range(M):
        idxg = np.zeros((K * CU,), np.int32)
        idxs = np.zeros((K * CU,), np.int32)
        cnt = np.zeros((K * CU,), np.float32)
        bsum = np.zeros((K * CU, E), np.float32)
        for k in range(K):
            uniq, cnts, bsums = per_bin[c * K + k]
            # pad with an unused segment row: scatter rewrites it unchanged
            free = np.setdiff1d(np.arange(uniq.size + 1, dtype=np.int64), uniq)[0]
            o = k * CU
            idxs[o : o + CU] = free
            idxs[o : o + uniq.size] = uniq
            idxg[o : o + CU] = idxs[o : o + CU] + k * RS
            cnt[o : o + uniq.size] = cnts
            bsum[o : o + uniq.size] = bsums
        in_maps.append(
            {
                "idxg_in": np.ascontiguousarray(idxg.reshape(K * T, P).T),
                "idxs_in": np.ascontiguousarray(idxs.reshape(K * T, P).T),
                "cnt_in": np.ascontiguousarray(cnt.reshape(K * T, P).T),
                "bsum_in": np.ascontiguousarray(
                    bsum.reshape(K * T, P, E).transpose(1, 0, 2).reshape(P, K * T * E)
                ),
            }
        )
    return T, in_maps


def kernel(y, batch, centers):
    y = np.asarray(y)
    batch = np.ascontiguousarray(np.asarray(batch, dtype=np.float32))
    centers = np.ascontiguousarray(np.asarray(centers, dtype=np.float32))
    y64 = y.astype(np.int64)

    T, in_maps = _route(y64, batch)
    for c in range(M):
        in_maps[c]["centers_in"] = centers[c * R : (c + 1) * R]
        in_maps[c]["bslice_in"] = batch[c * BS : (c + 1) * BS]

    nc = _BUILD_CACHE.get(T)
    if nc is None:
        nc = _build(T)
        _BUILD_CACHE[T] = nc

    res = run_bass_kernel_spmd(nc, in_maps, list(range(M))).results

    new_centers = np.concatenate(
        [res[c][f"out{k}"] for c in range(M) for k in range(K)], axis=0
    )
    total = np.float64(0.0)
    for c in range(M):
        total += np.asarray(res[c]["loss_out"], dtype=np.float64).sum()
    loss = np.asarray(LAMBDA * total / B, dtype=np.float32)
    return loss, new_centers
